# revision 19
# baseline (speedup 1.0000x reference)
"""Trainium2 Bass kernel for nn_BlockTransformerMixer.

Model: B=8, T=8192, D=256, H=4 heads (hd=64), L=2 layers, block size BS=16.
Block-local attention (block-diagonal over 16-token blocks).

Sharding: pure data parallel - core i processes batch element i (8192 tokens);
tiny layer weights replicated to all 8 cores. Full inputs in, full output out.

Per-core dataflow (token-major resident fp32 x in SBUF; bf16 matmul inputs,
fp32 PSUM accumulation; weights pre-transposed host-side with norm weights and
1/sqrt(hd) folded in):
  per layer: attention phase (ACT set: ln/exp), barrier, ffn phase (gelu) -
  phase split keeps ACT table loads to 4 for the whole kernel.
  attention per 512-token super-tile:
    s1 = rsqrt(mean(x^2)+eps) via exp(-0.5*ln(.)); xn = x*s1 (per-partition
    scalar); PE-transpose -> xnT (d-major); qkT = Wqk-stationary matmuls;
    V = xnT-stationary matmuls (token-major)
    per 128-token subtile (8 blocks):
      S^T_h = kT_h.T @ qT_h per head into its own PSUM bank; exp reads each
      bank directly (E batched [128, 4*128]); E_m = E * blockmask (DVE);
      AV with ones-augmented V rhs: o_h|rowsum_h = E_m_h.T @ [V_h|1]
      (token-major, so the softmax normalizer lands as a per-partition
      column); recip = 1/rowsums on [128,4] (cheap: DVE free-dim serial);
      o = o_unnorm * recip (folded into the PSUM-evacuation copy)
    o -> PE-transpose -> oT; aoutT = Wo-stationary matmuls; transpose back;
    x += a (fp32 residual); squared-sum stats stashed for the next norm.
  ffn: xn2 -> transpose -> ff1 (W1-stationary, ap=512) -> gelu (psum->sbuf)
    -> ff2 (8-step k-accumulation) -> transpose -> x += m.

Container-specific workarounds (walrus "b16 cc-2026-05-04"):
  - at most ONE sync wait per instruction: _split_excess_waits moves excess
    waits onto injected same-engine NoOps placed just before the instruction
  - custom-DVE ops (tensor_tensor_reduce, reciprocal_approx_*) do not lower:
    use square+reduce_sum and plain reciprocal on small tiles instead
  - every matmul accumulation group must write its own PSUM tile starting at
    offset 0 (sub-bank column offsets or multiple groups per bank fault at
    execution time); transposes and per-head scores each get a private tile
"""

import math
import os
from contextlib import ExitStack

import numpy as np
import ml_dtypes

B, T, D = 8, 8192, 256
H, L, BS = 4, 2, 16
HD = D // H
EPS = 1e-6
P = 128
N_CORES = 8

_BUILD_CACHE = {}


def _np_bf16(a):
    return np.asarray(a, dtype=np.float32).astype(ml_dtypes.bfloat16)


def _split_excess_waits(nc, max_waits=1):
    """The walrus in this container encodes at most one sync wait per
    instruction ("Too many sync wait commands" otherwise). Tile attaches up to
    a handful. Split the excess onto injected same-engine NoOps placed
    immediately before the instruction (sequencers execute in order, so the
    semantics are identical)."""
    import bass_rust
    import concourse.mybir as mybir

    n_split = 0
    for bb in nc.main_func.blocks:
        insts = bb.instructions
        out = []
        changed = False
        for inst in insts:
            si = inst.sync_info
            waits = list(si.on_wait) if si is not None else []
            if len(waits) > max_waits:
                keep = waits[-max_waits:]
                extra = waits[:-max_waits]
                for k, w in enumerate(extra):
                    nop = mybir.InstNoOp(
                        name=f"{inst.name}-wsplit{k}",
                        engine=inst.engine,
                        ins=[],
                        outs=[],
                        sync_info=bass_rust.SyncInfo(on_wait=[w], on_update=[]),
                    )
                    try:
                        nc.register_instruction(nop, overwrite=True)
                    except Exception:
                        pass
                    out.append(nop)
                inst.sync_info = bass_rust.SyncInfo(
                    on_wait=keep, on_update=list(si.on_update)
                )
                n_split += 1
                changed = True
            out.append(inst)
        if changed:
            insts[:] = out
    return n_split


DEFAULT_EM = {
    "stats": "v1",     # v1: DVE fused sq+accum; s1: Act Square+accum (Pool ~1us/op on HW: avoid)
    "xn_scale": "v",   # per-token rms scale of x
    "tp_evac": "vv",   # xn transpose evacuations (engine per d-half)
    "qkT_evac": "v",
    "vbf_evac": "v",
    "sums": "v",       # rowsum column gathers
    "o_evac": "ssvv",  # per-head scaled PSUM evacuation of o
    "ot_evac": "vs",
    "aT_evac": "vs",
    "res_add": "vv",   # residual evac-adds (engine per e-chunk)
    "f2_evac": "vs",
}


def build_nc(tokens=T, em=None, bufs_work=4, bufs_stw=3, bufs_big=3, bufs_small=2):
    """Build the Bass module for one core processing `tokens` tokens."""
    import concourse.bass as bass
    import concourse.mybir as mybir
    import concourse.tile as tile
    from concourse.bass import ts

    f32 = mybir.dt.float32
    bf16 = mybir.dt.bfloat16
    AF = mybir.ActivationFunctionType
    OP = mybir.AluOpType

    em = dict(DEFAULT_EM, **(em or {}))

    NSUB = tokens // P          # 128-token subtiles
    STW = 4                     # subtiles per super-tile
    NST = NSUB // STW           # super-tiles (512 tokens each)
    assert NST * STW == NSUB

    nc = bass.Bass()

    def _psum(*aps):
        # GPSIMD (Pool) cannot access PSUM on HW (BIR verifier rejects it,
        # though CoreSim accepts it) - fall back to DVE for those.
        return any(ap.space == bass.MemorySpace.PSUM for ap in aps)

    def _copy(eng, dst, src):
        if eng == "g" and _psum(dst, src):
            eng = "v"
        if eng == "s":
            nc.scalar.copy(dst, src)
        elif eng == "g":
            nc.gpsimd.tensor_copy(dst, src)
        else:
            nc.vector.tensor_copy(dst, src)

    def _scaled_copy(eng, dst, src, scale_ap):
        if eng == "g" and _psum(dst, src):
            eng = "v"
        if eng == "s":
            nc.scalar.activation(dst, src, AF.Copy, scale=scale_ap)
        elif eng == "g":
            nc.gpsimd.tensor_scalar_mul(dst, src, scale_ap)
        else:
            nc.vector.tensor_scalar_mul(dst, src, scale_ap)

    def _add(eng, dst, a, b, tmp_pool=None):
        # "p": Act evacuates the PSUM operand to SBUF, Pool does the add -
        # spreads residual work onto the otherwise PSUM-banned Pool engine.
        if eng == "p" and tmp_pool is not None and _psum(b):
            shp = [b.partition_size(), b.free_size()]
            tmp = tmp_pool.tile(shp, mybir.dt.bfloat16, tag="addtmp",
                                name="addtmp")
            nc.scalar.copy(tmp[:], b)
            nc.gpsimd.tensor_tensor(dst, a, tmp[:], OP.add)
            return
        if eng == "g" and _psum(dst, a, b):
            eng = "v"
        e = nc.gpsimd if eng == "g" else nc.vector
        e.tensor_tensor(dst, a, b, OP.add)

    x_in = nc.declare_dram_parameter("x", [tokens, D], f32, isOutput=False)
    wqk_d = nc.declare_dram_parameter("wqk", [P, L, 2, 4, P], bf16, isOutput=False)
    wv_d = nc.declare_dram_parameter("wv", [P, L, 2, D], bf16, isOutput=False)
    wo_d = nc.declare_dram_parameter("wo", [P, L, 2, D], bf16, isOutput=False)
    w1_d = nc.declare_dram_parameter("w1", [P, L, 2, 4 * D], bf16, isOutput=False)
    w2_d = nc.declare_dram_parameter("w2", [P, L, 8, D], bf16, isOutput=False)
    mk_d = nc.declare_dram_parameter("mk", [9, 4, 4 * P], bf16, isOutput=False)
    mq_d = nc.declare_dram_parameter("mq", [9, 4, 4 * P], bf16, isOutput=False)
    ident_d = nc.declare_dram_parameter("ident", [P, P], bf16, isOutput=False)
    out_d = nc.declare_dram_parameter("out", [tokens, D], f32, isOutput=True)

    x_t = x_in.rearrange("(a p) d -> p a d", p=P)
    out_t = out_d.rearrange("(a p) d -> p a d", p=P)

    with tile.TileContext(nc) as tc, ExitStack() as ctx:
        persist = ctx.enter_context(tc.tile_pool(name="persist", bufs=1))
        work = ctx.enter_context(tc.tile_pool(name="work", bufs=bufs_work))
        stw = ctx.enter_context(tc.tile_pool(name="stwork", bufs=bufs_stw))
        ps = ctx.enter_context(tc.tile_pool(name="ps", bufs=2, space="PSUM"))
        ps2 = ctx.enter_context(tc.tile_pool(name="ps2", bufs=bufs_small, space="PSUM"))
        ps3 = ctx.enter_context(tc.tile_pool(name="ps3", bufs=4, space="PSUM"))

        # ---- persistent tiles ----
        x_sb = persist.tile([P, NSUB, D], f32, tag="x_sb")
        wqk_sb = persist.tile([P, L, 2, 4, P], bf16, tag="wqk")
        wv_sb = persist.tile([P, L, 2, D], bf16, tag="wv")
        wo_sb = persist.tile([P, L, 2, D], bf16, tag="wo")
        w1_sb = persist.tile([P, L, 2, 4 * D], bf16, tag="w1")
        w2_sb = persist.tile([P, L, 8, D], bf16, tag="w2")
        ident_sb = persist.tile([P, P], bf16, tag="ident")
        # augmented q/k tiles: rows 0:64 = per-head qT/kT (rewritten per super
        # tile); rows 64:73 = additive-mask factor rows (-C*1*1^T + C*U*U^T
        # fused into the score matmul's contraction; softmax shift-invariance
        # cancels the bf16 rounding of sqrt(C)). Manual 3-deep ring.
        kaug_bufs = [persist.tile([73, 4, STW * P], bf16, tag=f"kaug{i}",
                                  name=f"kaug{i}") for i in range(3)]
        qaug_bufs = [persist.tile([73, 4, STW * P], bf16, tag=f"qaug{i}",
                                  name=f"qaug{i}") for i in range(3)]
        msA_sb = persist.tile([P, NSUB], f32, tag="msA")
        msB_sb = persist.tile([P, NSUB], f32, tag="msB")
        sA_sb = persist.tile([P, NSUB], f32, tag="sA")
        sB_sb = persist.tile([P, NSUB], f32, tag="sB")
        lntmp_sb = persist.tile([P, NSUB], f32, tag="lntmp")
        eps_sb = persist.tile([P, 1], f32, tag="eps")
        nc.gpsimd.memset(eps_sb[:], EPS)

        # ---- DMA order: masks/ident first (transposes need ident), then x
        # chunks interleaved with weights so layer-0 can start early ----
        def stash_sq_stats(src_ap, ms_ap):
            # custom-DVE ops (tensor_tensor_reduce) don't lower in this
            # container's walrus. g1/s1: single fused square + free-dim
            # accumulate; v2 fallback: square then reduce (2 DVE ops).
            sq = work.tile([P, D], bf16, tag="sq")
            if em["stats"] == "g2v":
                # Pool does the (SBUF-only) square, DVE the reduce. Pool can't
                # run scalar_tensor_tensor / tensor_reduce in this walrus.
                nc.gpsimd.tensor_tensor(sq[:], src_ap, src_ap, OP.mult)
                nc.vector.reduce_sum(ms_ap, sq[:], axis=mybir.AxisListType.X)
            elif em["stats"] == "v1":
                nc.vector.scalar_tensor_tensor(
                    sq[:], src_ap, 1.0, src_ap, OP.bypass, OP.mult,
                    accum_out=ms_ap)
            elif em["stats"] == "s1":
                nc.scalar.activation(sq[:], src_ap, AF.Square, accum_out=ms_ap)
            else:
                nc.vector.tensor_tensor(sq[:], src_ap, src_ap, OP.mult)
                nc.vector.reduce_sum(ms_ap, sq[:], axis=mybir.AxisListType.X)

        def rms_st(ms, s_out, st):
            # s = exp(-0.5 * ln(ms/D + eps)) = rsqrt(mean_sq + eps), for one
            # super-tile (phase-global rms would join on every subtile's stats)
            sl = slice(st * STW, (st + 1) * STW)
            nc.scalar.activation(lntmp_sb[:, sl], ms[:, sl],
                                 AF.Ln, bias=eps_sb[:, 0:1], scale=1.0 / D)
            nc.scalar.activation(s_out[:, sl], lntmp_sb[:, sl], AF.Exp,
                                 scale=-0.5)

        nc.sync.dma_start(ident_sb[:], ident_d[:])
        for i in range(3):
            nc.sync.dma_start(kaug_bufs[i][64:73, :, :], mk_d[:])
            nc.sync.dma_start(qaug_bufs[i][64:73, :, :], mq_d[:])
        wdmas = [(wqk_sb, wqk_d), (wv_sb, wv_d), (wo_sb, wo_d),
                 (w1_sb, w1_d), (w2_sb, w2_d)]
        for st in range(NST):
            sl = slice(st * STW, (st + 1) * STW)
            nc.sync.dma_start(x_sb[:, sl, :], x_t[:, sl, :])
            if wdmas:
                sb, d = wdmas.pop(0)
                nc.sync.dma_start(sb[:], d[:])
            for s4 in range(STW):
                s = st * STW + s4
                stash_sq_stats(x_sb[:, s, :], msA_sb[:, s : s + 1])
            rms_st(msA_sb, sA_sb, st)
        for sb, d in wdmas:
            nc.sync.dma_start(sb[:], d[:])

        def transpose_pair(dst_bf, src_sb, s4):
            # src_sb [P, 256] (token-major) -> dst_bf[:, dh, s4*128:...] (d-major)
            # each transpose gets its own psum tile (HW: one matmul group per
            # bank, output at tile offset 0 only)
            for dh in range(2):
                tp = ps2.tile([P, P], bf16, tag="small", name="tp")
                nc.tensor.transpose(tp[:], src_sb[:, ts(dh, P)], ident_sb[:])
                _copy(em["tp_evac"][dh], dst_bf[:, dh, ts(s4, P)], tp[:])

        # ---- pipelined stage emitters (in-order engines: emission order IS
        # the per-engine schedule; stages of super-tile st+1 are interleaved
        # into st's stall windows) ----
        xnT_map, oT_map = {}, {}

        def emit_A(st, s_vec):
            # norm-scale + transpose: xnT(st)
            xnT = stw.tile([P, 2, STW * P], bf16, tag="xnT")
            xnT_map[st] = xnT
            for s4 in range(STW):
                s = st * STW + s4
                xn = work.tile([P, D], bf16, tag="xn")
                _scaled_copy(em["xn_scale"], xn[:], x_sb[:, s, :],
                             s_vec[:, s : s + 1])
                transpose_pair(xnT, xn, s4)

        def emit_Qgrp(st, l, ec):
            # one qkT e-chunk projection group; evacuate the two 64-row head
            # halves into the augmented q/k tiles (q: chunks 0-1, k: 2-3)
            qk_ps = ps.tile([P, STW * P], f32, tag="big", name="qk_ps")
            for dh in range(2):
                nc.tensor.matmul(
                    qk_ps[:], wqk_sb[:, l, dh, ec, :], xnT_map[st][:, dh, :],
                    start=(dh == 0), stop=(dh == 1),
                )
            dst = qaug_bufs[st % 3] if ec < 2 else kaug_bufs[st % 3]
            for hh in range(2):
                _copy(em["qkT_evac"], dst[0:64, (ec % 2) * 2 + hh, :],
                      qk_ps[64 * hh : 64 * (hh + 1), :])

        def emit_V(st, l, s4):
            # V token-major [128 tok, 256] with appended ones column per head:
            # AV then yields the per-(head,q) masked-E row sums as an extra col.
            v_ps = ps2.tile([P, D], f32, tag="small", name="v_ps")
            for dh in range(2):
                nc.tensor.matmul(
                    v_ps[:], xnT_map[st][:, dh, ts(s4, P)], wv_sb[:, l, dh, :],
                    start=(dh == 0), stop=(dh == 1),
                )
            v_bf = work.tile([P, 4, 65], bf16, tag="v_bf")
            _copy(em["vbf_evac"], v_bf[:, :, 0:64],
                  v_ps[:].rearrange("p (h e) -> p h e", h=4))
            nc.gpsimd.memset(v_bf[:, :, 64:65], 1.0)
            return v_bf

        def emit_sc(st, l, s4):
            # scores (mask folded into contraction) + exp for one subtile
            kaug = kaug_bufs[st % 3]
            qaug = qaug_bufs[st % 3]
            e_bf = work.tile([P, 4 * P], bf16, tag="e_bf")
            sh_tiles = []
            for h in range(4):
                sh_ps = ps3.tile([P, P], f32, tag="sth", name="sh_ps")
                nc.tensor.matmul(
                    sh_ps[:], kaug[0:73, h, ts(s4, P)], qaug[0:73, h, ts(s4, P)],
                    start=True, stop=True,
                )
                sh_tiles.append(sh_ps)
            for h in range(4):
                nc.scalar.activation(e_bf[:, ts(h, P)], sh_tiles[h][:], AF.Exp)
            return e_bf

        def emit_av(st, l, s4, e_bf, v_bf):
            # AV + per-head softmax-normalize + transpose to d-major oT
            oT = oT_map[st]
            o_tok = work.tile([P, D], bf16, tag="o_tok")
            recip_tm = work.tile([P, 4], f32, tag="recip_tm")
            for h in range(4):
                oh_ps = ps3.tile([P, 65], f32, tag="sth", name="oh_ps")
                nc.tensor.matmul(
                    oh_ps[:], e_bf[:, ts(h, P)], v_bf[:, h, :],
                    start=True, stop=True,
                )
                nc.vector.reciprocal(recip_tm[:, h : h + 1], oh_ps[:, 64:65])
                _scaled_copy(em["o_evac"][h], o_tok[:, ts(h, 64)],
                             oh_ps[:, 0:64], recip_tm[:, h : h + 1])
            for dh in range(2):
                ot_ps = ps2.tile([P, P], bf16, tag="small", name="ot_ps")
                nc.tensor.transpose(ot_ps[:], o_tok[:, ts(dh, P)], ident_sb[:])
                _copy(em["ot_evac"][dh], oT[:, dh, ts(s4, P)], ot_ps[:])

        def emit_O(st, l, ms_next):
            # out-proj (d-major) + transpose-back + residual + stats
            oT = oT_map[st]
            aT = stw.tile([P, 2, STW * P], bf16, tag="aT")
            for ec in range(2):
                aT_ps = ps.tile([P, STW * P], f32, tag="big", name="aT_ps")
                for dh in range(2):
                    nc.tensor.matmul(
                        aT_ps[:], wo_sb[:, l, dh, ts(ec, P)], oT[:, dh, :],
                        start=(dh == 0), stop=(dh == 1),
                    )
                _copy(em["aT_evac"][ec], aT[:, ec, :], aT_ps[:])
            for s4 in range(STW):
                s = st * STW + s4
                for ec in range(2):
                    a_ps = ps2.tile([P, P], bf16, tag="small", name="a_ps")
                    nc.tensor.transpose(a_ps[:], aT[:, ec, ts(s4, P)],
                                        ident_sb[:])
                    _add(em["res_add"][ec], x_sb[:, s, ts(ec, P)],
                         x_sb[:, s, ts(ec, P)], a_ps[:], tmp_pool=work)
                stash_sq_stats(x_sb[:, s, :], ms_next[:, s : s + 1])

        m1_map = {}

        def emit_F(st, l):
            # ff1 + gelu -> m1(st)
            m1 = stw.tile([P, 8, STW * P], bf16, tag="m1", name="m1")
            m1_map[st] = m1
            for fc in range(8):
                f1_ps = ps.tile([P, STW * P], f32, tag="big", name="f1_ps")
                for dh in range(2):
                    nc.tensor.matmul(
                        f1_ps[:], w1_sb[:, l, dh, ts(fc, P)],
                        xnT_map[st][:, dh, :],
                        start=(dh == 0), stop=(dh == 1),
                    )
                nc.scalar.activation(m1[:, fc, :], f1_ps[:], AF.Gelu)

        def emit_G(st, l, ms_next):
            # ff2 + transpose-back + residual (+ stats for next layer)
            m1 = m1_map[st]
            a2T = stw.tile([P, 2, STW * P], bf16, tag="aT")
            for ec in range(2):
                f2_ps = ps.tile([P, STW * P], f32, tag="big", name="f2_ps")
                for fc in range(8):
                    nc.tensor.matmul(
                        f2_ps[:], w2_sb[:, l, fc, ts(ec, P)], m1[:, fc, :],
                        start=(fc == 0), stop=(fc == 7),
                    )
                _copy(em["f2_evac"][ec], a2T[:, ec, :], f2_ps[:])
            for s4 in range(STW):
                s = st * STW + s4
                for ec in range(2):
                    a_ps = ps2.tile([P, P], bf16, tag="small", name="a_ps")
                    nc.tensor.transpose(a_ps[:], a2T[:, ec, ts(s4, P)],
                                        ident_sb[:])
                    _add(em["res_add"][ec], x_sb[:, s, ts(ec, P)],
                         x_sb[:, s, ts(ec, P)], a_ps[:], tmp_pool=work)
                if ms_next is not None:
                    stash_sq_stats(x_sb[:, s, :], ms_next[:, s : s + 1])
            if ms_next is None:
                sl = slice(st * STW, (st + 1) * STW)
                nc.sync.dma_start(out_t[:, sl, :], x_sb[:, sl, :])

        for l in range(L):
            ms_attn, s_attn = (msA_sb, sA_sb)
            ms_ffn, s_ffn = (msB_sb, sB_sb)
            # ======== attention phase ========
            # s_attn: layer 0's comes from the load loop; layer l+1's is
            # emitted per-st here (Ln/Exp live in this phase's act table).
            if l > 0:
                rms_st(ms_attn, s_attn, 0)
            emit_A(0, s_attn)
            for ec in range(4):
                emit_Qgrp(0, l, ec)
            for st in range(NST):
                if st + 1 < NST:
                    if l > 0:
                        rms_st(ms_attn, s_attn, st + 1)
                    emit_A(st + 1, s_attn)
                oT_map[st] = stw.tile([P, 2, STW * P], bf16, tag="oT", name="oT")
                # scores run one subtile ahead of AV: exp latency never gates
                # the AV matmuls, and the score psum ring slot frees at exp
                vb = emit_V(st, l, 0)
                eb = emit_sc(st, l, 0)
                for s4 in range(STW):
                    vb2 = emit_V(st, l, s4 + 1) if s4 + 1 < STW else None
                    eb2 = emit_sc(st, l, s4 + 1) if s4 + 1 < STW else None
                    emit_av(st, l, s4, eb, vb)
                    if st + 1 < NST:
                        emit_Qgrp(st + 1, l, s4)
                    vb, eb = vb2, eb2
                emit_O(st, l, ms_ffn)
                # s_ffn(st) right after this st's stats: still the exp table
                rms_st(ms_ffn, s_ffn, st)
            tc.no_sync_barrier()
            # ======== ffn phase ========
            ms_next = msA_sb if l + 1 < L else None
            emit_A(0, s_ffn)
            emit_F(0, l)
            for st in range(NST):
                if st + 1 < NST:
                    emit_A(st + 1, s_ffn)
                emit_G(st, l, ms_next)
                if st + 1 < NST:
                    emit_F(st + 1, l)
            if l + 1 < L:
                tc.no_sync_barrier()

    _split_excess_waits(nc)
    return nc


def prep_aux(norm1_w, in_proj_w, out_proj_w, norm2_w, ff1_w, ff2_w):
    """Host-side weight layout prep (all lhsT layouts for d-on-partition matmuls)."""
    ipw = np.asarray(in_proj_w, np.float32) * np.asarray(norm1_w, np.float32)[:, None, :]
    ipw = ipw.copy()
    ipw[:, :D, :] *= 1.0 / math.sqrt(HD)  # fold score scale into W_q
    wqk = np.empty((L, 2, 4, P, P), np.float32)
    wv = np.empty((L, 2, P, D), np.float32)
    wo = np.empty((L, 2, P, D), np.float32)
    w1 = np.empty((L, 2, P, 4 * D), np.float32)
    w2 = np.empty((L, 8, P, D), np.float32)
    for l in range(L):
        wt = ipw[l, : 2 * D, :].T  # [256 d, 512 e(qk)]
        for dh in range(2):
            for ec in range(4):
                wqk[l, dh, ec] = wt[dh * P : (dh + 1) * P, ec * P : (ec + 1) * P]
        vt = ipw[l, 2 * D :, :].T  # [256 d, 256 e]
        ot = np.asarray(out_proj_w[l], np.float32).T  # [256 d, 256 e]
        f1t = (np.asarray(ff1_w[l], np.float32)
               * np.asarray(norm2_w[l], np.float32)[None, :]).T  # [256 d, 1024 f]
        f2t = np.asarray(ff2_w[l], np.float32).T  # [1024 f, 256 e]
        for dh in range(2):
            wv[l, dh] = vt[dh * P : (dh + 1) * P, :]
            wo[l, dh] = ot[dh * P : (dh + 1) * P, :]
            w1[l, dh] = f1t[dh * P : (dh + 1) * P, :]
        for fc in range(8):
            w2[l, fc] = f2t[fc * P : (fc + 1) * P, :]
    ident = np.eye(P, dtype=np.float32)
    # additive-mask factor rows: M = -C*1*1^T + C*U*U^T (U = 16-token block
    # indicator); fused into the score matmul as 9 extra contraction rows.
    # bf16 rounding of sqrt(C) only shifts on-block scores by a constant,
    # which the softmax normalization cancels exactly.
    C = 30.0
    U = np.zeros((P, 8), np.float32)
    for i in range(P):
        U[i, i // BS] = 1.0
    sq = np.asarray(np.sqrt(C), np.float32)
    kc = np.concatenate([np.full((1, P), -C, np.float32), sq * U.T], axis=0)  # [9,128]
    qc = np.concatenate([np.ones((1, P), np.float32), sq * U.T], axis=0)
    mk = np.broadcast_to(np.tile(kc, (1, 4))[:, None, :], (9, 4, 4 * P)).copy()
    mq = np.broadcast_to(np.tile(qc, (1, 4))[:, None, :], (9, 4, 4 * P)).copy()
    # partition-major DRAM layouts: SBUF-destination order, 1 desc/partition
    wqk_p = np.ascontiguousarray(np.transpose(wqk, (3, 0, 1, 2, 4)))
    wv_p = np.ascontiguousarray(np.transpose(wv, (2, 0, 1, 3)))
    wo_p = np.ascontiguousarray(np.transpose(wo, (2, 0, 1, 3)))
    w1_p = np.ascontiguousarray(np.transpose(w1, (2, 0, 1, 3)))
    w2_p = np.ascontiguousarray(np.transpose(w2, (2, 0, 1, 3)))
    return {
        "wqk": _np_bf16(wqk_p), "wv": _np_bf16(wv_p), "wo": _np_bf16(wo_p),
        "w1": _np_bf16(w1_p), "w2": _np_bf16(w2_p),
        "mk": _np_bf16(mk), "mq": _np_bf16(mq), "ident": _np_bf16(ident),
    }


def kernel(h, norm1_w, in_proj_w, in_proj_b, out_proj_w, out_proj_b,
           norm2_w, ff1_w, ff1_b, ff2_w, ff2_b):
    from concourse.bass_utils import run_bass_kernel_spmd

    h = np.asarray(h, np.float32)
    aux = prep_aux(norm1_w, in_proj_w, out_proj_w, norm2_w, ff1_w, ff2_w)

    key = ("nc", T)
    if key not in _BUILD_CACHE:
        _BUILD_CACHE[key] = build_nc(T)
    nc = _BUILD_CACHE[key]

    in_maps = []
    for c in range(N_CORES):
        m = {"x": np.ascontiguousarray(h[c])}
        m.update(aux)
        in_maps.append(m)

    res = run_bass_kernel_spmd(nc, in_maps, list(range(N_CORES)),
                               trace=bool(int(os.environ.get("KERNEL_TRACE", "0"))))
    if res.exec_time_ns is not None:
        kernel.last_exec_time_ns = res.exec_time_ns
    out = np.stack([res.results[c]["out"] for c in range(N_CORES)], axis=0)
    return out


kernel.last_exec_time_ns = None



# revision 21
# speedup vs baseline: 1.0262x; 1.0262x over previous
"""Trainium2 Bass kernel for nn_BlockTransformerMixer.

Model: B=8, T=8192, D=256, H=4 heads (hd=64), L=2 layers, block size BS=16.
Block-local attention (block-diagonal over 16-token blocks).

Sharding: pure data parallel - core i processes batch element i (8192 tokens);
tiny layer weights replicated to all 8 cores. Full inputs in, full output out.

Per-core dataflow (token-major resident fp32 x in SBUF; bf16 matmul inputs,
fp32 PSUM accumulation; weights pre-transposed host-side with norm weights and
1/sqrt(hd) folded in):
  per layer: attention phase (ACT set: ln/exp), barrier, ffn phase (gelu) -
  phase split keeps ACT table loads to 4 for the whole kernel.
  attention per 512-token super-tile:
    s1 = rsqrt(mean(x^2)+eps) via exp(-0.5*ln(.)); xn = x*s1 (per-partition
    scalar); PE-transpose -> xnT (d-major); qkT = Wqk-stationary matmuls;
    V = xnT-stationary matmuls (token-major)
    per 128-token subtile (8 blocks):
      S^T_h = kT_h.T @ qT_h per head into its own PSUM bank; exp reads each
      bank directly (E batched [128, 4*128]); E_m = E * blockmask (DVE);
      AV with ones-augmented V rhs: o_h|rowsum_h = E_m_h.T @ [V_h|1]
      (token-major, so the softmax normalizer lands as a per-partition
      column); recip = 1/rowsums on [128,4] (cheap: DVE free-dim serial);
      o = o_unnorm * recip (folded into the PSUM-evacuation copy)
    o -> PE-transpose -> oT; aoutT = Wo-stationary matmuls; transpose back;
    x += a (fp32 residual); squared-sum stats stashed for the next norm.
  ffn: xn2 -> transpose -> ff1 (W1-stationary, ap=512) -> gelu (psum->sbuf)
    -> ff2 (8-step k-accumulation) -> transpose -> x += m.

Container-specific workarounds (walrus "b16 cc-2026-05-04"):
  - at most ONE sync wait per instruction: _split_excess_waits moves excess
    waits onto injected same-engine NoOps placed just before the instruction
  - custom-DVE ops (tensor_tensor_reduce, reciprocal_approx_*) do not lower:
    use square+reduce_sum and plain reciprocal on small tiles instead
  - every matmul accumulation group must write its own PSUM tile starting at
    offset 0 (sub-bank column offsets or multiple groups per bank fault at
    execution time); transposes and per-head scores each get a private tile
"""

import math
import os
from contextlib import ExitStack

import numpy as np
import ml_dtypes

B, T, D = 8, 8192, 256
H, L, BS = 4, 2, 16
HD = D // H
EPS = 1e-6
P = 128
N_CORES = 8

_BUILD_CACHE = {}


def _np_bf16(a):
    return np.asarray(a, dtype=np.float32).astype(ml_dtypes.bfloat16)


def _split_excess_waits(nc, max_waits=1):
    """The walrus in this container encodes at most one sync wait per
    instruction ("Too many sync wait commands" otherwise). Tile attaches up to
    a handful. Split the excess onto injected same-engine NoOps placed
    immediately before the instruction (sequencers execute in order, so the
    semantics are identical)."""
    import bass_rust
    import concourse.mybir as mybir

    n_split = 0
    for bb in nc.main_func.blocks:
        insts = bb.instructions
        out = []
        changed = False
        for inst in insts:
            si = inst.sync_info
            waits = list(si.on_wait) if si is not None else []
            if len(waits) > max_waits:
                keep = waits[-max_waits:]
                extra = waits[:-max_waits]
                for k, w in enumerate(extra):
                    nop = mybir.InstNoOp(
                        name=f"{inst.name}-wsplit{k}",
                        engine=inst.engine,
                        ins=[],
                        outs=[],
                        sync_info=bass_rust.SyncInfo(on_wait=[w], on_update=[]),
                    )
                    try:
                        nc.register_instruction(nop, overwrite=True)
                    except Exception:
                        pass
                    out.append(nop)
                inst.sync_info = bass_rust.SyncInfo(
                    on_wait=keep, on_update=list(si.on_update)
                )
                n_split += 1
                changed = True
            out.append(inst)
        if changed:
            insts[:] = out
    return n_split


DEFAULT_EM = {
    "stats": "v1",     # v1: DVE fused sq+accum; s1: Act Square+accum (Pool ~1us/op on HW: avoid)
    "xn_scale": "v",   # per-token rms scale of x
    "tp_evac": "vv",   # xn transpose evacuations (engine per d-half)
    "qkT_evac": "v",
    "vbf_evac": "v",
    "sums": "v",       # rowsum column gathers
    "o_evac": "ssvv",  # per-head scaled PSUM evacuation of o
    "ot_evac": "vs",
    "aT_evac": "vs",
    "res_add": "vv",   # residual evac-adds (engine per e-chunk)
    "f2_evac": "vs",
    "nobar": 0,        # 1: drop inter-phase no_sync_barriers
    "xdt": "f32",      # x residual-stream dtype: f32 | bf16
}


def build_nc(tokens=T, em=None, bufs_work=4, bufs_stw=3, bufs_big=3, bufs_small=2):
    """Build the Bass module for one core processing `tokens` tokens."""
    import concourse.bass as bass
    import concourse.mybir as mybir
    import concourse.tile as tile
    from concourse.bass import ts

    f32 = mybir.dt.float32
    bf16 = mybir.dt.bfloat16
    AF = mybir.ActivationFunctionType
    OP = mybir.AluOpType

    em = dict(DEFAULT_EM, **(em or {}))

    NSUB = tokens // P          # 128-token subtiles
    STW = 4                     # subtiles per super-tile
    NST = NSUB // STW           # super-tiles (512 tokens each)
    assert NST * STW == NSUB

    nc = bass.Bass()

    def _psum(*aps):
        # GPSIMD (Pool) cannot access PSUM on HW (BIR verifier rejects it,
        # though CoreSim accepts it) - fall back to DVE for those.
        return any(ap.space == bass.MemorySpace.PSUM for ap in aps)

    def _copy(eng, dst, src):
        if eng == "g" and _psum(dst, src):
            eng = "v"
        if eng == "s":
            nc.scalar.copy(dst, src)
        elif eng == "g":
            nc.gpsimd.tensor_copy(dst, src)
        else:
            nc.vector.tensor_copy(dst, src)

    def _scaled_copy(eng, dst, src, scale_ap):
        if eng == "g" and _psum(dst, src):
            eng = "v"
        if eng == "s":
            nc.scalar.activation(dst, src, AF.Copy, scale=scale_ap)
        elif eng == "g":
            nc.gpsimd.tensor_scalar_mul(dst, src, scale_ap)
        else:
            nc.vector.tensor_scalar_mul(dst, src, scale_ap)

    def _add(eng, dst, a, b, tmp_pool=None):
        # "p": Act evacuates the PSUM operand to SBUF, Pool does the add -
        # spreads residual work onto the otherwise PSUM-banned Pool engine.
        if eng == "p" and tmp_pool is not None and _psum(b):
            shp = [b.partition_size(), b.free_size()]
            tmp = tmp_pool.tile(shp, mybir.dt.bfloat16, tag="addtmp",
                                name="addtmp")
            nc.scalar.copy(tmp[:], b)
            nc.gpsimd.tensor_tensor(dst, a, tmp[:], OP.add)
            return
        if eng == "g" and _psum(dst, a, b):
            eng = "v"
        e = nc.gpsimd if eng == "g" else nc.vector
        e.tensor_tensor(dst, a, b, OP.add)

    x_in = nc.declare_dram_parameter("x", [tokens, D], f32, isOutput=False)
    wqk_d = nc.declare_dram_parameter("wqk", [P, L, 2, 4, P], bf16, isOutput=False)
    wv_d = nc.declare_dram_parameter("wv", [P, L, 2, D], bf16, isOutput=False)
    wo_d = nc.declare_dram_parameter("wo", [P, L, 2, D], bf16, isOutput=False)
    w1_d = nc.declare_dram_parameter("w1", [P, L, 2, 4 * D], bf16, isOutput=False)
    w2_d = nc.declare_dram_parameter("w2", [P, L, 8, D], bf16, isOutput=False)
    mk_d = nc.declare_dram_parameter("mk", [9, 4, 4 * P], bf16, isOutput=False)
    mq_d = nc.declare_dram_parameter("mq", [9, 4, 4 * P], bf16, isOutput=False)
    ident_d = nc.declare_dram_parameter("ident", [P, P], bf16, isOutput=False)
    out_d = nc.declare_dram_parameter("out", [tokens, D], f32, isOutput=True)

    x_t = x_in.rearrange("(a p) d -> p a d", p=P)
    out_t = out_d.rearrange("(a p) d -> p a d", p=P)

    with tile.TileContext(nc) as tc, ExitStack() as ctx:
        persist = ctx.enter_context(tc.tile_pool(name="persist", bufs=1))
        work = ctx.enter_context(tc.tile_pool(name="work", bufs=bufs_work))
        stw = ctx.enter_context(tc.tile_pool(name="stwork", bufs=bufs_stw))
        ps = ctx.enter_context(tc.tile_pool(name="ps", bufs=2, space="PSUM"))
        ps2 = ctx.enter_context(tc.tile_pool(name="ps2", bufs=bufs_small, space="PSUM"))
        ps3 = ctx.enter_context(tc.tile_pool(name="ps3", bufs=4, space="PSUM"))

        # ---- persistent tiles ----
        xdt = bf16 if em["xdt"] == "bf16" else f32
        x_sb = persist.tile([P, NSUB, D], xdt, tag="x_sb")
        wqk_sb = persist.tile([P, L, 2, 4, P], bf16, tag="wqk")
        wv_sb = persist.tile([P, L, 2, D], bf16, tag="wv")
        wo_sb = persist.tile([P, L, 2, D], bf16, tag="wo")
        w1_sb = persist.tile([P, L, 2, 4 * D], bf16, tag="w1")
        w2_sb = persist.tile([P, L, 8, D], bf16, tag="w2")
        ident_sb = persist.tile([P, P], bf16, tag="ident")
        # augmented q/k tiles: rows 0:64 = per-head qT/kT (rewritten per super
        # tile); rows 64:73 = additive-mask factor rows (-C*1*1^T + C*U*U^T
        # fused into the score matmul's contraction; softmax shift-invariance
        # cancels the bf16 rounding of sqrt(C)). Manual 3-deep ring.
        kaug_bufs = [persist.tile([73, 4, STW * P], bf16, tag=f"kaug{i}",
                                  name=f"kaug{i}") for i in range(3)]
        qaug_bufs = [persist.tile([73, 4, STW * P], bf16, tag=f"qaug{i}",
                                  name=f"qaug{i}") for i in range(3)]
        msA_sb = persist.tile([P, NSUB], f32, tag="msA")
        msB_sb = persist.tile([P, NSUB], f32, tag="msB")
        sA_sb = persist.tile([P, NSUB], f32, tag="sA")
        sB_sb = persist.tile([P, NSUB], f32, tag="sB")
        lntmp_sb = persist.tile([P, NSUB], f32, tag="lntmp")
        eps_sb = persist.tile([P, 1], f32, tag="eps")
        nc.gpsimd.memset(eps_sb[:], EPS)

        # ---- DMA order: masks/ident first (transposes need ident), then x
        # chunks interleaved with weights so layer-0 can start early ----
        def stash_sq_stats(src_ap, ms_ap):
            # custom-DVE ops (tensor_tensor_reduce) don't lower in this
            # container's walrus. g1/s1: single fused square + free-dim
            # accumulate; v2 fallback: square then reduce (2 DVE ops).
            sq = work.tile([P, D], bf16, tag="sq")
            if em["stats"] == "g2v":
                # Pool does the (SBUF-only) square, DVE the reduce. Pool can't
                # run scalar_tensor_tensor / tensor_reduce in this walrus.
                nc.gpsimd.tensor_tensor(sq[:], src_ap, src_ap, OP.mult)
                nc.vector.reduce_sum(ms_ap, sq[:], axis=mybir.AxisListType.X)
            elif em["stats"] == "v1":
                nc.vector.scalar_tensor_tensor(
                    sq[:], src_ap, 1.0, src_ap, OP.bypass, OP.mult,
                    accum_out=ms_ap)
            elif em["stats"] == "s1":
                nc.scalar.activation(sq[:], src_ap, AF.Square, accum_out=ms_ap)
            else:
                nc.vector.tensor_tensor(sq[:], src_ap, src_ap, OP.mult)
                nc.vector.reduce_sum(ms_ap, sq[:], axis=mybir.AxisListType.X)

        def rms_st(ms, s_out, st):
            # s = exp(-0.5 * ln(ms/D + eps)) = rsqrt(mean_sq + eps), for one
            # super-tile (phase-global rms would join on every subtile's stats)
            sl = slice(st * STW, (st + 1) * STW)
            nc.scalar.activation(lntmp_sb[:, sl], ms[:, sl],
                                 AF.Ln, bias=eps_sb[:, 0:1], scale=1.0 / D)
            nc.scalar.activation(s_out[:, sl], lntmp_sb[:, sl], AF.Exp,
                                 scale=-0.5)

        nc.sync.dma_start(ident_sb[:], ident_d[:])
        for i in range(3):
            nc.sync.dma_start(kaug_bufs[i][64:73, :, :], mk_d[:])
            nc.sync.dma_start(qaug_bufs[i][64:73, :, :], mq_d[:])
        wdmas = [(wqk_sb, wqk_d), (wv_sb, wv_d), (wo_sb, wo_d),
                 (w1_sb, w1_d), (w2_sb, w2_d)]
        for st in range(NST):
            sl = slice(st * STW, (st + 1) * STW)
            if em["xdt"] == "bf16":
                # only gpsimd-initiated DMAs can cast f32 dram -> bf16 sbuf
                nc.gpsimd.dma_start(x_sb[:, sl, :], x_t[:, sl, :])
            else:
                nc.sync.dma_start(x_sb[:, sl, :], x_t[:, sl, :])
            if wdmas:
                sb, d = wdmas.pop(0)
                nc.sync.dma_start(sb[:], d[:])
            for s4 in range(STW):
                s = st * STW + s4
                stash_sq_stats(x_sb[:, s, :], msA_sb[:, s : s + 1])
            rms_st(msA_sb, sA_sb, st)
        for sb, d in wdmas:
            nc.sync.dma_start(sb[:], d[:])

        def transpose_pair(dst_bf, src_sb, s4):
            # src_sb [P, 256] (token-major) -> dst_bf[:, dh, s4*128:...] (d-major)
            # each transpose gets its own psum tile (HW: one matmul group per
            # bank, output at tile offset 0 only)
            for dh in range(2):
                tp = ps2.tile([P, P], bf16, tag="small", name="tp")
                nc.tensor.transpose(tp[:], src_sb[:, ts(dh, P)], ident_sb[:])
                _copy(em["tp_evac"][dh], dst_bf[:, dh, ts(s4, P)], tp[:])

        # ---- pipelined stage emitters (in-order engines: emission order IS
        # the per-engine schedule; stages of super-tile st+1 are interleaved
        # into st's stall windows) ----
        xnT_map, oT_map = {}, {}

        def emit_A(st, s_vec):
            # norm-scale + transpose: xnT(st)
            xnT = stw.tile([P, 2, STW * P], bf16, tag="xnT")
            xnT_map[st] = xnT
            for s4 in range(STW):
                s = st * STW + s4
                xn = work.tile([P, D], bf16, tag="xn")
                _scaled_copy(em["xn_scale"], xn[:], x_sb[:, s, :],
                             s_vec[:, s : s + 1])
                transpose_pair(xnT, xn, s4)

        def emit_Qgrp(st, l, ec):
            # one qkT e-chunk projection group; evacuate the two 64-row head
            # halves into the augmented q/k tiles (q: chunks 0-1, k: 2-3)
            qk_ps = ps.tile([P, STW * P], f32, tag="big", name="qk_ps")
            for dh in range(2):
                nc.tensor.matmul(
                    qk_ps[:], wqk_sb[:, l, dh, ec, :], xnT_map[st][:, dh, :],
                    start=(dh == 0), stop=(dh == 1),
                )
            dst = qaug_bufs[st % 3] if ec < 2 else kaug_bufs[st % 3]
            for hh in range(2):
                _copy(em["qkT_evac"], dst[0:64, (ec % 2) * 2 + hh, :],
                      qk_ps[64 * hh : 64 * (hh + 1), :])

        def emit_V(st, l, s4):
            # V token-major [128 tok, 256] with appended ones column per head:
            # AV then yields the per-(head,q) masked-E row sums as an extra col.
            v_ps = ps2.tile([P, D], f32, tag="small", name="v_ps")
            for dh in range(2):
                nc.tensor.matmul(
                    v_ps[:], xnT_map[st][:, dh, ts(s4, P)], wv_sb[:, l, dh, :],
                    start=(dh == 0), stop=(dh == 1),
                )
            v_bf = work.tile([P, 4, 65], bf16, tag="v_bf")
            _copy(em["vbf_evac"], v_bf[:, :, 0:64],
                  v_ps[:].rearrange("p (h e) -> p h e", h=4))
            nc.gpsimd.memset(v_bf[:, :, 64:65], 1.0)
            return v_bf

        def emit_sc(st, l, s4):
            # scores (mask folded into contraction) + exp for one subtile
            kaug = kaug_bufs[st % 3]
            qaug = qaug_bufs[st % 3]
            e_bf = work.tile([P, 4 * P], bf16, tag="e_bf")
            sh_tiles = []
            for h in range(4):
                sh_ps = ps3.tile([P, P], f32, tag="sth", name="sh_ps")
                nc.tensor.matmul(
                    sh_ps[:], kaug[0:73, h, ts(s4, P)], qaug[0:73, h, ts(s4, P)],
                    start=True, stop=True,
                )
                sh_tiles.append(sh_ps)
            for h in range(4):
                nc.scalar.activation(e_bf[:, ts(h, P)], sh_tiles[h][:], AF.Exp)
            return e_bf

        def emit_av(st, l, s4, e_bf, v_bf):
            # AV + per-head softmax-normalize + transpose to d-major oT
            oT = oT_map[st]
            o_tok = work.tile([P, D], bf16, tag="o_tok")
            recip_tm = work.tile([P, 4], f32, tag="recip_tm")
            for h in range(4):
                oh_ps = ps3.tile([P, 65], f32, tag="sth", name="oh_ps")
                nc.tensor.matmul(
                    oh_ps[:], e_bf[:, ts(h, P)], v_bf[:, h, :],
                    start=True, stop=True,
                )
                nc.vector.reciprocal(recip_tm[:, h : h + 1], oh_ps[:, 64:65])
                _scaled_copy(em["o_evac"][h], o_tok[:, ts(h, 64)],
                             oh_ps[:, 0:64], recip_tm[:, h : h + 1])
            for dh in range(2):
                ot_ps = ps2.tile([P, P], bf16, tag="small", name="ot_ps")
                nc.tensor.transpose(ot_ps[:], o_tok[:, ts(dh, P)], ident_sb[:])
                _copy(em["ot_evac"][dh], oT[:, dh, ts(s4, P)], ot_ps[:])

        def emit_O(st, l, ms_next):
            # out-proj (d-major) + transpose-back + residual + stats
            oT = oT_map[st]
            aT = stw.tile([P, 2, STW * P], bf16, tag="aT")
            for ec in range(2):
                aT_ps = ps.tile([P, STW * P], f32, tag="big", name="aT_ps")
                for dh in range(2):
                    nc.tensor.matmul(
                        aT_ps[:], wo_sb[:, l, dh, ts(ec, P)], oT[:, dh, :],
                        start=(dh == 0), stop=(dh == 1),
                    )
                _copy(em["aT_evac"][ec], aT[:, ec, :], aT_ps[:])
            for s4 in range(STW):
                s = st * STW + s4
                for ec in range(2):
                    a_ps = ps2.tile([P, P], bf16, tag="small", name="a_ps")
                    nc.tensor.transpose(a_ps[:], aT[:, ec, ts(s4, P)],
                                        ident_sb[:])
                    _add(em["res_add"][ec], x_sb[:, s, ts(ec, P)],
                         x_sb[:, s, ts(ec, P)], a_ps[:], tmp_pool=work)
                stash_sq_stats(x_sb[:, s, :], ms_next[:, s : s + 1])

        m1_map = {}

        def emit_F(st, l):
            # ff1 + gelu -> m1(st)
            m1 = stw.tile([P, 8, STW * P], bf16, tag="m1", name="m1")
            m1_map[st] = m1
            for fc in range(8):
                f1_ps = ps.tile([P, STW * P], f32, tag="big", name="f1_ps")
                for dh in range(2):
                    nc.tensor.matmul(
                        f1_ps[:], w1_sb[:, l, dh, ts(fc, P)],
                        xnT_map[st][:, dh, :],
                        start=(dh == 0), stop=(dh == 1),
                    )
                nc.scalar.activation(m1[:, fc, :], f1_ps[:], AF.Gelu)

        def emit_G(st, l, ms_next):
            # ff2 + transpose-back + residual (+ stats for next layer)
            m1 = m1_map[st]
            a2T = stw.tile([P, 2, STW * P], bf16, tag="aT")
            for ec in range(2):
                f2_ps = ps.tile([P, STW * P], f32, tag="big", name="f2_ps")
                for fc in range(8):
                    nc.tensor.matmul(
                        f2_ps[:], w2_sb[:, l, fc, ts(ec, P)], m1[:, fc, :],
                        start=(fc == 0), stop=(fc == 7),
                    )
                _copy(em["f2_evac"][ec], a2T[:, ec, :], f2_ps[:])
            for s4 in range(STW):
                s = st * STW + s4
                for ec in range(2):
                    a_ps = ps2.tile([P, P], bf16, tag="small", name="a_ps")
                    nc.tensor.transpose(a_ps[:], a2T[:, ec, ts(s4, P)],
                                        ident_sb[:])
                    _add(em["res_add"][ec], x_sb[:, s, ts(ec, P)],
                         x_sb[:, s, ts(ec, P)], a_ps[:], tmp_pool=work)
                if ms_next is not None:
                    stash_sq_stats(x_sb[:, s, :], ms_next[:, s : s + 1])
            if ms_next is None:
                sl = slice(st * STW, (st + 1) * STW)
                if em["xdt"] == "bf16":
                    xo = stw.tile([P, STW, D], f32, tag="xo", name="xo")
                    nc.vector.tensor_copy(xo[:], x_sb[:, sl, :])
                    nc.sync.dma_start(out_t[:, sl, :], xo[:])
                else:
                    nc.sync.dma_start(out_t[:, sl, :], x_sb[:, sl, :])

        for l in range(L):
            ms_attn, s_attn = (msA_sb, sA_sb)
            ms_ffn, s_ffn = (msB_sb, sB_sb)
            # ======== attention phase ========
            # s_attn: layer 0's comes from the load loop; layer l+1's is
            # emitted per-st here (Ln/Exp live in this phase's act table).
            if l > 0:
                rms_st(ms_attn, s_attn, 0)
            emit_A(0, s_attn)
            for ec in range(4):
                emit_Qgrp(0, l, ec)
            for st in range(NST):
                if st + 1 < NST:
                    if l > 0:
                        rms_st(ms_attn, s_attn, st + 1)
                    emit_A(st + 1, s_attn)
                oT_map[st] = stw.tile([P, 2, STW * P], bf16, tag="oT", name="oT")
                # scores run one subtile ahead of AV: exp latency never gates
                # the AV matmuls, and the score psum ring slot frees at exp
                vb = emit_V(st, l, 0)
                eb = emit_sc(st, l, 0)
                for s4 in range(STW):
                    vb2 = emit_V(st, l, s4 + 1) if s4 + 1 < STW else None
                    eb2 = emit_sc(st, l, s4 + 1) if s4 + 1 < STW else None
                    emit_av(st, l, s4, eb, vb)
                    if st + 1 < NST:
                        emit_Qgrp(st + 1, l, s4)
                    vb, eb = vb2, eb2
                emit_O(st, l, ms_ffn)
                # s_ffn(st) right after this st's stats: still the exp table
                rms_st(ms_ffn, s_ffn, st)
            if not em["nobar"]:
                tc.no_sync_barrier()
            # ======== ffn phase ========
            ms_next = msA_sb if l + 1 < L else None
            emit_A(0, s_ffn)
            emit_F(0, l)
            for st in range(NST):
                if st + 1 < NST:
                    emit_A(st + 1, s_ffn)
                emit_G(st, l, ms_next)
                if st + 1 < NST:
                    emit_F(st + 1, l)
            if l + 1 < L and not em["nobar"]:
                tc.no_sync_barrier()

    _split_excess_waits(nc)
    return nc


def prep_aux(norm1_w, in_proj_w, out_proj_w, norm2_w, ff1_w, ff2_w):
    """Host-side weight layout prep (all lhsT layouts for d-on-partition matmuls)."""
    ipw = np.asarray(in_proj_w, np.float32) * np.asarray(norm1_w, np.float32)[:, None, :]
    ipw = ipw.copy()
    ipw[:, :D, :] *= 1.0 / math.sqrt(HD)  # fold score scale into W_q
    wqk = np.empty((L, 2, 4, P, P), np.float32)
    wv = np.empty((L, 2, P, D), np.float32)
    wo = np.empty((L, 2, P, D), np.float32)
    w1 = np.empty((L, 2, P, 4 * D), np.float32)
    w2 = np.empty((L, 8, P, D), np.float32)
    for l in range(L):
        wt = ipw[l, : 2 * D, :].T  # [256 d, 512 e(qk)]
        for dh in range(2):
            for ec in range(4):
                wqk[l, dh, ec] = wt[dh * P : (dh + 1) * P, ec * P : (ec + 1) * P]
        vt = ipw[l, 2 * D :, :].T  # [256 d, 256 e]
        ot = np.asarray(out_proj_w[l], np.float32).T  # [256 d, 256 e]
        f1t = (np.asarray(ff1_w[l], np.float32)
               * np.asarray(norm2_w[l], np.float32)[None, :]).T  # [256 d, 1024 f]
        f2t = np.asarray(ff2_w[l], np.float32).T  # [1024 f, 256 e]
        for dh in range(2):
            wv[l, dh] = vt[dh * P : (dh + 1) * P, :]
            wo[l, dh] = ot[dh * P : (dh + 1) * P, :]
            w1[l, dh] = f1t[dh * P : (dh + 1) * P, :]
        for fc in range(8):
            w2[l, fc] = f2t[fc * P : (fc + 1) * P, :]
    ident = np.eye(P, dtype=np.float32)
    # additive-mask factor rows: M = -C*1*1^T + C*U*U^T (U = 16-token block
    # indicator); fused into the score matmul as 9 extra contraction rows.
    # bf16 rounding of sqrt(C) only shifts on-block scores by a constant,
    # which the softmax normalization cancels exactly.
    C = 30.0
    U = np.zeros((P, 8), np.float32)
    for i in range(P):
        U[i, i // BS] = 1.0
    sq = np.asarray(np.sqrt(C), np.float32)
    kc = np.concatenate([np.full((1, P), -C, np.float32), sq * U.T], axis=0)  # [9,128]
    qc = np.concatenate([np.ones((1, P), np.float32), sq * U.T], axis=0)
    mk = np.broadcast_to(np.tile(kc, (1, 4))[:, None, :], (9, 4, 4 * P)).copy()
    mq = np.broadcast_to(np.tile(qc, (1, 4))[:, None, :], (9, 4, 4 * P)).copy()
    # partition-major DRAM layouts: SBUF-destination order, 1 desc/partition
    wqk_p = np.ascontiguousarray(np.transpose(wqk, (3, 0, 1, 2, 4)))
    wv_p = np.ascontiguousarray(np.transpose(wv, (2, 0, 1, 3)))
    wo_p = np.ascontiguousarray(np.transpose(wo, (2, 0, 1, 3)))
    w1_p = np.ascontiguousarray(np.transpose(w1, (2, 0, 1, 3)))
    w2_p = np.ascontiguousarray(np.transpose(w2, (2, 0, 1, 3)))
    return {
        "wqk": _np_bf16(wqk_p), "wv": _np_bf16(wv_p), "wo": _np_bf16(wo_p),
        "w1": _np_bf16(w1_p), "w2": _np_bf16(w2_p),
        "mk": _np_bf16(mk), "mq": _np_bf16(mq), "ident": _np_bf16(ident),
    }


def kernel(h, norm1_w, in_proj_w, in_proj_b, out_proj_w, out_proj_b,
           norm2_w, ff1_w, ff1_b, ff2_w, ff2_b):
    from concourse.bass_utils import run_bass_kernel_spmd

    h = np.asarray(h, np.float32)
    aux = prep_aux(norm1_w, in_proj_w, out_proj_w, norm2_w, ff1_w, ff2_w)

    key = ("nc", T)
    if key not in _BUILD_CACHE:
        _BUILD_CACHE[key] = build_nc(T)
    nc = _BUILD_CACHE[key]

    in_maps = []
    for c in range(N_CORES):
        m = {"x": np.ascontiguousarray(h[c])}
        m.update(aux)
        in_maps.append(m)

    res = run_bass_kernel_spmd(nc, in_maps, list(range(N_CORES)),
                               trace=bool(int(os.environ.get("KERNEL_TRACE", "0"))))
    if res.exec_time_ns is not None:
        kernel.last_exec_time_ns = res.exec_time_ns
    out = np.stack([res.results[c]["out"] for c in range(N_CORES)], axis=0)
    return out


kernel.last_exec_time_ns = None



# revision 22
# speedup vs baseline: 1.3183x; 1.2847x over previous
"""Trainium2 Bass kernel for nn_BlockTransformerMixer.

Model: B=8, T=8192, D=256, H=4 heads (hd=64), L=2 layers, block size BS=16.
Block-local attention (block-diagonal over 16-token blocks).

Sharding: pure data parallel - core i processes batch element i (8192 tokens);
tiny layer weights replicated to all 8 cores. Full inputs in, full output out.

Per-core dataflow (token-major resident fp32 x in SBUF; bf16 matmul inputs,
fp32 PSUM accumulation; weights pre-transposed host-side with norm weights and
1/sqrt(hd) folded in):
  per layer: attention phase (ACT set: ln/exp), barrier, ffn phase (gelu) -
  phase split keeps ACT table loads to 4 for the whole kernel.
  attention per 512-token super-tile:
    s1 = rsqrt(mean(x^2)+eps) via exp(-0.5*ln(.)); xn = x*s1 (per-partition
    scalar); PE-transpose -> xnT (d-major); qkT = Wqk-stationary matmuls;
    V = xnT-stationary matmuls (token-major)
    per 128-token subtile (8 blocks):
      S^T_h = kT_h.T @ qT_h per head into its own PSUM bank; exp reads each
      bank directly (E batched [128, 4*128]); E_m = E * blockmask (DVE);
      AV with ones-augmented V rhs: o_h|rowsum_h = E_m_h.T @ [V_h|1]
      (token-major, so the softmax normalizer lands as a per-partition
      column); recip = 1/rowsums on [128,4] (cheap: DVE free-dim serial);
      o = o_unnorm * recip (folded into the PSUM-evacuation copy)
    o -> PE-transpose -> oT; aoutT = Wo-stationary matmuls; transpose back;
    x += a (fp32 residual); squared-sum stats stashed for the next norm.
  ffn: xn2 -> transpose -> ff1 (W1-stationary, ap=512) -> gelu (psum->sbuf)
    -> ff2 (8-step k-accumulation) -> transpose -> x += m.

Container-specific workarounds (walrus "b16 cc-2026-05-04"):
  - at most ONE sync wait per instruction: _split_excess_waits moves excess
    waits onto injected same-engine NoOps placed just before the instruction
  - custom-DVE ops (tensor_tensor_reduce, reciprocal_approx_*) do not lower:
    use square+reduce_sum and plain reciprocal on small tiles instead
  - every matmul accumulation group must write its own PSUM tile starting at
    offset 0 (sub-bank column offsets or multiple groups per bank fault at
    execution time); transposes and per-head scores each get a private tile
"""

import math
import os
from contextlib import ExitStack

import numpy as np
import ml_dtypes

B, T, D = 8, 8192, 256
H, L, BS = 4, 2, 16
HD = D // H
EPS = 1e-6
P = 128
N_CORES = 8

_BUILD_CACHE = {}


def _np_bf16(a):
    return np.asarray(a, dtype=np.float32).astype(ml_dtypes.bfloat16)


def _split_excess_waits(nc, max_waits=1):
    """The walrus in this container encodes at most one sync wait per
    instruction ("Too many sync wait commands" otherwise). Tile attaches up to
    a handful. Split the excess onto injected same-engine NoOps placed
    immediately before the instruction (sequencers execute in order, so the
    semantics are identical)."""
    import bass_rust
    import concourse.mybir as mybir

    n_split = 0
    for bb in nc.main_func.blocks:
        insts = bb.instructions
        out = []
        changed = False
        for inst in insts:
            si = inst.sync_info
            waits = list(si.on_wait) if si is not None else []
            if len(waits) > max_waits:
                keep = waits[-max_waits:]
                extra = waits[:-max_waits]
                for k, w in enumerate(extra):
                    nop = mybir.InstNoOp(
                        name=f"{inst.name}-wsplit{k}",
                        engine=inst.engine,
                        ins=[],
                        outs=[],
                        sync_info=bass_rust.SyncInfo(on_wait=[w], on_update=[]),
                    )
                    try:
                        nc.register_instruction(nop, overwrite=True)
                    except Exception:
                        pass
                    out.append(nop)
                inst.sync_info = bass_rust.SyncInfo(
                    on_wait=keep, on_update=list(si.on_update)
                )
                n_split += 1
                changed = True
            out.append(inst)
        if changed:
            insts[:] = out
    return n_split


DEFAULT_EM = {
    "stats": "v1",     # v1: DVE fused sq+accum; s1: Act Square+accum (Pool ~1us/op on HW: avoid)
    "xn_scale": "v",   # per-token rms scale of x
    "tp_evac": "vv",   # xn transpose evacuations (engine per d-half)
    "qkT_evac": "v",
    "vbf_evac": "v",
    "sums": "v",       # rowsum column gathers
    "o_evac": "ssvv",  # per-head scaled PSUM evacuation of o
    "ot_evac": "vs",
    "aT_evac": "vs",
    "res_add": "vv",   # residual evac-adds (engine per e-chunk)
    "f2_evac": "vs",
    "nobar": 0,        # 1: drop inter-phase no_sync_barriers
    "skip_attn": 0,    # ablation: zero the attention subtile stream
    "skip_ffn": 0,     # ablation: drop ff1/ff2
    "xdt": "f32",      # x residual-stream dtype: f32 | bf16
}


def build_nc(tokens=T, em=None, bufs_work=6, bufs_stw=4, bufs_big=3, bufs_small=2):
    """Build the Bass module for one core processing `tokens` tokens."""
    import concourse.bass as bass
    import concourse.mybir as mybir
    import concourse.tile as tile
    from concourse.bass import ts

    f32 = mybir.dt.float32
    bf16 = mybir.dt.bfloat16
    AF = mybir.ActivationFunctionType
    OP = mybir.AluOpType

    em = dict(DEFAULT_EM, **(em or {}))

    NSUB = tokens // P          # 128-token subtiles
    STW = 4                     # subtiles per super-tile
    NST = NSUB // STW           # super-tiles (512 tokens each)
    assert NST * STW == NSUB

    nc = bass.Bass()

    def _psum(*aps):
        # GPSIMD (Pool) cannot access PSUM on HW (BIR verifier rejects it,
        # though CoreSim accepts it) - fall back to DVE for those.
        return any(ap.space == bass.MemorySpace.PSUM for ap in aps)

    def _copy(eng, dst, src):
        if eng == "g" and _psum(dst, src):
            eng = "v"
        if eng == "s":
            nc.scalar.copy(dst, src)
        elif eng == "g":
            nc.gpsimd.tensor_copy(dst, src)
        else:
            nc.vector.tensor_copy(dst, src)

    def _scaled_copy(eng, dst, src, scale_ap):
        if eng == "g" and _psum(dst, src):
            eng = "v"
        if eng == "s":
            nc.scalar.activation(dst, src, AF.Copy, scale=scale_ap)
        elif eng == "g":
            nc.gpsimd.tensor_scalar_mul(dst, src, scale_ap)
        else:
            nc.vector.tensor_scalar_mul(dst, src, scale_ap)

    def _add(eng, dst, a, b, tmp_pool=None):
        # "p": Act evacuates the PSUM operand to SBUF, Pool does the add -
        # spreads residual work onto the otherwise PSUM-banned Pool engine.
        if eng == "p" and tmp_pool is not None and _psum(b):
            shp = [b.partition_size(), b.free_size()]
            tmp = tmp_pool.tile(shp, mybir.dt.bfloat16, tag="addtmp",
                                name="addtmp")
            nc.scalar.copy(tmp[:], b)
            nc.gpsimd.tensor_tensor(dst, a, tmp[:], OP.add)
            return
        if eng == "g" and _psum(dst, a, b):
            eng = "v"
        e = nc.gpsimd if eng == "g" else nc.vector
        e.tensor_tensor(dst, a, b, OP.add)

    x_in = nc.declare_dram_parameter("x", [tokens, D], f32, isOutput=False)
    wqk_d = nc.declare_dram_parameter("wqk", [P, L, 2, 4, P], bf16, isOutput=False)
    wv_d = nc.declare_dram_parameter("wv", [P, L, 2, D], bf16, isOutput=False)
    wo_d = nc.declare_dram_parameter("wo", [P, L, 2, D], bf16, isOutput=False)
    w1_d = nc.declare_dram_parameter("w1", [P, L, 2, 4 * D], bf16, isOutput=False)
    w2_d = nc.declare_dram_parameter("w2", [P, L, 8, D], bf16, isOutput=False)
    mk_d = nc.declare_dram_parameter("mk", [9, 4, 4 * P], bf16, isOutput=False)
    mq_d = nc.declare_dram_parameter("mq", [9, 4, 4 * P], bf16, isOutput=False)
    ident_d = nc.declare_dram_parameter("ident", [P, P], bf16, isOutput=False)
    out_d = nc.declare_dram_parameter("out", [tokens, D], f32, isOutput=True)

    x_t = x_in.rearrange("(a p) d -> p a d", p=P)
    out_t = out_d.rearrange("(a p) d -> p a d", p=P)

    with tile.TileContext(nc) as tc, ExitStack() as ctx:
        persist = ctx.enter_context(tc.tile_pool(name="persist", bufs=1))
        work = ctx.enter_context(tc.tile_pool(name="work", bufs=bufs_work))
        stw = ctx.enter_context(tc.tile_pool(name="stwork", bufs=bufs_stw))
        ps = ctx.enter_context(tc.tile_pool(name="ps", bufs=2, space="PSUM"))
        ps2 = ctx.enter_context(tc.tile_pool(name="ps2", bufs=bufs_small, space="PSUM"))
        ps3 = ctx.enter_context(tc.tile_pool(name="ps3", bufs=4, space="PSUM"))

        # ---- persistent tiles ----
        xdt = bf16 if em["xdt"] == "bf16" else f32
        x_sb = persist.tile([P, NSUB, D], xdt, tag="x_sb")
        wqk_sb = persist.tile([P, L, 2, 4, P], bf16, tag="wqk")
        wv_sb = persist.tile([P, L, 2, D], bf16, tag="wv")
        wo_sb = persist.tile([P, L, 2, D], bf16, tag="wo")
        w1_sb = persist.tile([P, L, 2, 4 * D], bf16, tag="w1")
        w2_sb = persist.tile([P, L, 8, D], bf16, tag="w2")
        ident_sb = persist.tile([P, P], bf16, tag="ident")
        # augmented q/k tiles: rows 0:64 = per-head qT/kT (rewritten per super
        # tile); rows 64:73 = additive-mask factor rows (-C*1*1^T + C*U*U^T
        # fused into the score matmul's contraction; softmax shift-invariance
        # cancels the bf16 rounding of sqrt(C)). Manual 3-deep ring.
        kaug_bufs = [persist.tile([73, 4, STW * P], bf16, tag=f"kaug{i}",
                                  name=f"kaug{i}") for i in range(3)]
        qaug_bufs = [persist.tile([73, 4, STW * P], bf16, tag=f"qaug{i}",
                                  name=f"qaug{i}") for i in range(3)]
        msA_sb = persist.tile([P, NSUB], f32, tag="msA")
        msB_sb = persist.tile([P, NSUB], f32, tag="msB")
        sA_sb = persist.tile([P, NSUB], f32, tag="sA")
        sB_sb = persist.tile([P, NSUB], f32, tag="sB")
        lntmp_sb = persist.tile([P, NSUB], f32, tag="lntmp")
        eps_sb = persist.tile([P, 1], f32, tag="eps")
        nc.gpsimd.memset(eps_sb[:], EPS)

        # ---- DMA order: masks/ident first (transposes need ident), then x
        # chunks interleaved with weights so layer-0 can start early ----
        def stash_sq_stats(src_ap, ms_ap):
            # custom-DVE ops (tensor_tensor_reduce) don't lower in this
            # container's walrus. g1/s1: single fused square + free-dim
            # accumulate; v2 fallback: square then reduce (2 DVE ops).
            sq = work.tile([P, D], bf16, tag="sq")
            if em["stats"] == "g2v":
                # Pool does the (SBUF-only) square, DVE the reduce. Pool can't
                # run scalar_tensor_tensor / tensor_reduce in this walrus.
                nc.gpsimd.tensor_tensor(sq[:], src_ap, src_ap, OP.mult)
                nc.vector.reduce_sum(ms_ap, sq[:], axis=mybir.AxisListType.X)
            elif em["stats"] == "v1":
                nc.vector.scalar_tensor_tensor(
                    sq[:], src_ap, 1.0, src_ap, OP.bypass, OP.mult,
                    accum_out=ms_ap)
            elif em["stats"] == "s1":
                nc.scalar.activation(sq[:], src_ap, AF.Square, accum_out=ms_ap)
            else:
                nc.vector.tensor_tensor(sq[:], src_ap, src_ap, OP.mult)
                nc.vector.reduce_sum(ms_ap, sq[:], axis=mybir.AxisListType.X)

        def rms_st(ms, s_out, st):
            # s = exp(-0.5 * ln(ms/D + eps)) = rsqrt(mean_sq + eps), for one
            # super-tile (phase-global rms would join on every subtile's stats)
            sl = slice(st * STW, (st + 1) * STW)
            nc.scalar.activation(lntmp_sb[:, sl], ms[:, sl],
                                 AF.Ln, bias=eps_sb[:, 0:1], scale=1.0 / D)
            nc.scalar.activation(s_out[:, sl], lntmp_sb[:, sl], AF.Exp,
                                 scale=-0.5)

        nc.sync.dma_start(ident_sb[:], ident_d[:])
        for i in range(3):
            nc.sync.dma_start(kaug_bufs[i][64:73, :, :], mk_d[:])
            nc.sync.dma_start(qaug_bufs[i][64:73, :, :], mq_d[:])
        wdmas = [(wqk_sb, wqk_d), (wv_sb, wv_d), (wo_sb, wo_d),
                 (w1_sb, w1_d), (w2_sb, w2_d)]
        for st in range(NST):
            sl = slice(st * STW, (st + 1) * STW)
            if em["xdt"] == "bf16":
                # only gpsimd-initiated DMAs can cast f32 dram -> bf16 sbuf
                nc.gpsimd.dma_start(x_sb[:, sl, :], x_t[:, sl, :])
            else:
                nc.sync.dma_start(x_sb[:, sl, :], x_t[:, sl, :])
            if wdmas:
                sb, d = wdmas.pop(0)
                nc.sync.dma_start(sb[:], d[:])
            for s4 in range(STW):
                s = st * STW + s4
                stash_sq_stats(x_sb[:, s, :], msA_sb[:, s : s + 1])
            rms_st(msA_sb, sA_sb, st)
        for sb, d in wdmas:
            nc.sync.dma_start(sb[:], d[:])

        def transpose_pair(dst_bf, src_sb, s4):
            # src_sb [P, 256] (token-major) -> dst_bf[:, dh, s4*128:...] (d-major)
            # each transpose gets its own psum tile (HW: one matmul group per
            # bank, output at tile offset 0 only)
            for dh in range(2):
                tp = ps2.tile([P, P], bf16, tag="small", name="tp")
                nc.tensor.transpose(tp[:], src_sb[:, ts(dh, P)], ident_sb[:])
                _copy(em["tp_evac"][dh], dst_bf[:, dh, ts(s4, P)], tp[:])

        # ---- pipelined stage emitters (in-order engines: emission order IS
        # the per-engine schedule; stages of super-tile st+1 are interleaved
        # into st's stall windows) ----
        xnT_map, oT_map = {}, {}

        def emit_A(st, s_vec):
            # norm-scale + transpose: xnT(st)
            xnT = stw.tile([P, 2, STW * P], bf16, tag="xnT")
            xnT_map[st] = xnT
            for s4 in range(STW):
                s = st * STW + s4
                xn = work.tile([P, D], bf16, tag="xn")
                _scaled_copy(em["xn_scale"], xn[:], x_sb[:, s, :],
                             s_vec[:, s : s + 1])
                transpose_pair(xnT, xn, s4)

        def emit_Qgrp(st, l, ec):
            # one qkT e-chunk projection group; evacuate the two 64-row head
            # halves into the augmented q/k tiles (q: chunks 0-1, k: 2-3)
            qk_ps = ps.tile([P, STW * P], f32, tag="big", name="qk_ps")
            for dh in range(2):
                nc.tensor.matmul(
                    qk_ps[:], wqk_sb[:, l, dh, ec, :], xnT_map[st][:, dh, :],
                    start=(dh == 0), stop=(dh == 1),
                )
            dst = qaug_bufs[st % 3] if ec < 2 else kaug_bufs[st % 3]
            for hh in range(2):
                _copy(em["qkT_evac"], dst[0:64, (ec % 2) * 2 + hh, :],
                      qk_ps[64 * hh : 64 * (hh + 1), :])

        def emit_V(st, l, s4):
            # V token-major [128 tok, 256] with appended ones column per head:
            # AV then yields the per-(head,q) masked-E row sums as an extra col.
            v_ps = ps2.tile([P, D], f32, tag="small", name="v_ps")
            for dh in range(2):
                nc.tensor.matmul(
                    v_ps[:], xnT_map[st][:, dh, ts(s4, P)], wv_sb[:, l, dh, :],
                    start=(dh == 0), stop=(dh == 1),
                )
            v_bf = work.tile([P, 4, 65], bf16, tag="v_bf")
            _copy(em["vbf_evac"], v_bf[:, :, 0:64],
                  v_ps[:].rearrange("p (h e) -> p h e", h=4))
            nc.gpsimd.memset(v_bf[:, :, 64:65], 1.0)
            return v_bf

        def emit_sc(st, l, s4):
            # scores (mask folded into contraction) + exp for one subtile
            kaug = kaug_bufs[st % 3]
            qaug = qaug_bufs[st % 3]
            e_bf = work.tile([P, 4 * P], bf16, tag="e_bf")
            sh_tiles = []
            for h in range(4):
                sh_ps = ps3.tile([P, P], f32, tag="sth", name="sh_ps")
                nc.tensor.matmul(
                    sh_ps[:], kaug[0:73, h, ts(s4, P)], qaug[0:73, h, ts(s4, P)],
                    start=True, stop=True,
                )
                sh_tiles.append(sh_ps)
            for h in range(4):
                nc.scalar.activation(e_bf[:, ts(h, P)], sh_tiles[h][:], AF.Exp)
            return e_bf

        def emit_av(st, l, s4, e_bf, v_bf):
            # AV + per-head softmax-normalize + transpose to d-major oT
            oT = oT_map[st]
            o_tok = work.tile([P, D], bf16, tag="o_tok")
            recip_tm = work.tile([P, 4], f32, tag="recip_tm")
            for h in range(4):
                oh_ps = ps3.tile([P, 65], f32, tag="sth", name="oh_ps")
                nc.tensor.matmul(
                    oh_ps[:], e_bf[:, ts(h, P)], v_bf[:, h, :],
                    start=True, stop=True,
                )
                nc.vector.reciprocal(recip_tm[:, h : h + 1], oh_ps[:, 64:65])
                _scaled_copy(em["o_evac"][h], o_tok[:, ts(h, 64)],
                             oh_ps[:, 0:64], recip_tm[:, h : h + 1])
            for dh in range(2):
                ot_ps = ps2.tile([P, P], bf16, tag="small", name="ot_ps")
                nc.tensor.transpose(ot_ps[:], o_tok[:, ts(dh, P)], ident_sb[:])
                _copy(em["ot_evac"][dh], oT[:, dh, ts(s4, P)], ot_ps[:])

        def emit_O(st, l, ms_next):
            # out-proj (d-major) + transpose-back + residual + stats
            oT = oT_map[st]
            aT = stw.tile([P, 2, STW * P], bf16, tag="aT")
            for ec in range(2):
                aT_ps = ps.tile([P, STW * P], f32, tag="big", name="aT_ps")
                for dh in range(2):
                    nc.tensor.matmul(
                        aT_ps[:], wo_sb[:, l, dh, ts(ec, P)], oT[:, dh, :],
                        start=(dh == 0), stop=(dh == 1),
                    )
                _copy(em["aT_evac"][ec], aT[:, ec, :], aT_ps[:])
            for s4 in range(STW):
                s = st * STW + s4
                for ec in range(2):
                    a_ps = ps2.tile([P, P], bf16, tag="small", name="a_ps")
                    nc.tensor.transpose(a_ps[:], aT[:, ec, ts(s4, P)],
                                        ident_sb[:])
                    _add(em["res_add"][ec], x_sb[:, s, ts(ec, P)],
                         x_sb[:, s, ts(ec, P)], a_ps[:], tmp_pool=work)
                stash_sq_stats(x_sb[:, s, :], ms_next[:, s : s + 1])

        m1_map = {}

        def emit_F(st, l):
            # ff1 + gelu -> m1(st)
            m1 = stw.tile([P, 8, STW * P], bf16, tag="m1", name="m1")
            m1_map[st] = m1
            for fc in range(8):
                f1_ps = ps.tile([P, STW * P], f32, tag="big", name="f1_ps")
                for dh in range(2):
                    nc.tensor.matmul(
                        f1_ps[:], w1_sb[:, l, dh, ts(fc, P)],
                        xnT_map[st][:, dh, :],
                        start=(dh == 0), stop=(dh == 1),
                    )
                nc.scalar.activation(m1[:, fc, :], f1_ps[:], AF.Gelu)

        def emit_G(st, l, ms_next):
            # ff2 + transpose-back + residual (+ stats for next layer)
            m1 = m1_map[st]
            a2T = stw.tile([P, 2, STW * P], bf16, tag="aT")
            for ec in range(2):
                f2_ps = ps.tile([P, STW * P], f32, tag="big", name="f2_ps")
                for fc in range(8):
                    nc.tensor.matmul(
                        f2_ps[:], w2_sb[:, l, fc, ts(ec, P)], m1[:, fc, :],
                        start=(fc == 0), stop=(fc == 7),
                    )
                _copy(em["f2_evac"][ec], a2T[:, ec, :], f2_ps[:])
            for s4 in range(STW):
                s = st * STW + s4
                for ec in range(2):
                    a_ps = ps2.tile([P, P], bf16, tag="small", name="a_ps")
                    nc.tensor.transpose(a_ps[:], a2T[:, ec, ts(s4, P)],
                                        ident_sb[:])
                    _add(em["res_add"][ec], x_sb[:, s, ts(ec, P)],
                         x_sb[:, s, ts(ec, P)], a_ps[:], tmp_pool=work)
                if ms_next is not None:
                    stash_sq_stats(x_sb[:, s, :], ms_next[:, s : s + 1])
            if ms_next is None:
                sl = slice(st * STW, (st + 1) * STW)
                if em["xdt"] == "bf16":
                    xo = stw.tile([P, STW, D], f32, tag="xo", name="xo")
                    nc.vector.tensor_copy(xo[:], x_sb[:, sl, :])
                    nc.sync.dma_start(out_t[:, sl, :], xo[:])
                else:
                    nc.sync.dma_start(out_t[:, sl, :], x_sb[:, sl, :])

        for l in range(L):
            ms_attn, s_attn = (msA_sb, sA_sb)
            ms_ffn, s_ffn = (msB_sb, sB_sb)
            # ======== attention phase ========
            # s_attn: layer 0's comes from the load loop; layer l+1's is
            # emitted per-st here (Ln/Exp live in this phase's act table).
            if l > 0:
                rms_st(ms_attn, s_attn, 0)
            emit_A(0, s_attn)
            for ec in range(4):
                emit_Qgrp(0, l, ec)
            for st in range(NST):
                if st + 1 < NST:
                    if l > 0:
                        rms_st(ms_attn, s_attn, st + 1)
                    emit_A(st + 1, s_attn)
                oT_map[st] = stw.tile([P, 2, STW * P], bf16, tag="oT", name="oT")
                if em["skip_attn"]:
                    nc.gpsimd.memset(oT_map[st][:], 0.0)
                    if st + 1 < NST:
                        for s4 in range(STW):
                            emit_Qgrp(st + 1, l, s4)
                else:
                    # scores run one subtile ahead of AV: exp latency never
                    # gates AV matmuls; the score psum slot frees at exp
                    vb = emit_V(st, l, 0)
                    eb = emit_sc(st, l, 0)
                    for s4 in range(STW):
                        vb2 = emit_V(st, l, s4 + 1) if s4 + 1 < STW else None
                        eb2 = emit_sc(st, l, s4 + 1) if s4 + 1 < STW else None
                        emit_av(st, l, s4, eb, vb)
                        if st + 1 < NST:
                            emit_Qgrp(st + 1, l, s4)
                        vb, eb = vb2, eb2
                emit_O(st, l, ms_ffn)
                # s_ffn(st) right after this st's stats: still the exp table
                rms_st(ms_ffn, s_ffn, st)
            if not em["nobar"]:
                tc.no_sync_barrier()
            # ======== ffn phase ========
            ms_next = msA_sb if l + 1 < L else None
            if em["skip_ffn"]:
                for st in range(NST):
                    for s4 in range(STW):
                        s = st * STW + s4
                        if ms_next is not None:
                            stash_sq_stats(x_sb[:, s, :], ms_next[:, s : s + 1])
                    if ms_next is None:
                        sl = slice(st * STW, (st + 1) * STW)
                        nc.sync.dma_start(out_t[:, sl, :], x_sb[:, sl, :])
            else:
                emit_A(0, s_ffn)
                emit_F(0, l)
                for st in range(NST):
                    if st + 1 < NST:
                        emit_A(st + 1, s_ffn)
                    emit_G(st, l, ms_next)
                    if st + 1 < NST:
                        emit_F(st + 1, l)
            if l + 1 < L and not em["nobar"]:
                tc.no_sync_barrier()

    _split_excess_waits(nc)
    return nc


def prep_aux(norm1_w, in_proj_w, out_proj_w, norm2_w, ff1_w, ff2_w):
    """Host-side weight layout prep (all lhsT layouts for d-on-partition matmuls)."""
    ipw = np.asarray(in_proj_w, np.float32) * np.asarray(norm1_w, np.float32)[:, None, :]
    ipw = ipw.copy()
    ipw[:, :D, :] *= 1.0 / math.sqrt(HD)  # fold score scale into W_q
    wqk = np.empty((L, 2, 4, P, P), np.float32)
    wv = np.empty((L, 2, P, D), np.float32)
    wo = np.empty((L, 2, P, D), np.float32)
    w1 = np.empty((L, 2, P, 4 * D), np.float32)
    w2 = np.empty((L, 8, P, D), np.float32)
    for l in range(L):
        wt = ipw[l, : 2 * D, :].T  # [256 d, 512 e(qk)]
        for dh in range(2):
            for ec in range(4):
                wqk[l, dh, ec] = wt[dh * P : (dh + 1) * P, ec * P : (ec + 1) * P]
        vt = ipw[l, 2 * D :, :].T  # [256 d, 256 e]
        ot = np.asarray(out_proj_w[l], np.float32).T  # [256 d, 256 e]
        f1t = (np.asarray(ff1_w[l], np.float32)
               * np.asarray(norm2_w[l], np.float32)[None, :]).T  # [256 d, 1024 f]
        f2t = np.asarray(ff2_w[l], np.float32).T  # [1024 f, 256 e]
        for dh in range(2):
            wv[l, dh] = vt[dh * P : (dh + 1) * P, :]
            wo[l, dh] = ot[dh * P : (dh + 1) * P, :]
            w1[l, dh] = f1t[dh * P : (dh + 1) * P, :]
        for fc in range(8):
            w2[l, fc] = f2t[fc * P : (fc + 1) * P, :]
    ident = np.eye(P, dtype=np.float32)
    # additive-mask factor rows: M = -C*1*1^T + C*U*U^T (U = 16-token block
    # indicator); fused into the score matmul as 9 extra contraction rows.
    # bf16 rounding of sqrt(C) only shifts on-block scores by a constant,
    # which the softmax normalization cancels exactly.
    C = 30.0
    U = np.zeros((P, 8), np.float32)
    for i in range(P):
        U[i, i // BS] = 1.0
    sq = np.asarray(np.sqrt(C), np.float32)
    kc = np.concatenate([np.full((1, P), -C, np.float32), sq * U.T], axis=0)  # [9,128]
    qc = np.concatenate([np.ones((1, P), np.float32), sq * U.T], axis=0)
    mk = np.broadcast_to(np.tile(kc, (1, 4))[:, None, :], (9, 4, 4 * P)).copy()
    mq = np.broadcast_to(np.tile(qc, (1, 4))[:, None, :], (9, 4, 4 * P)).copy()
    # partition-major DRAM layouts: SBUF-destination order, 1 desc/partition
    wqk_p = np.ascontiguousarray(np.transpose(wqk, (3, 0, 1, 2, 4)))
    wv_p = np.ascontiguousarray(np.transpose(wv, (2, 0, 1, 3)))
    wo_p = np.ascontiguousarray(np.transpose(wo, (2, 0, 1, 3)))
    w1_p = np.ascontiguousarray(np.transpose(w1, (2, 0, 1, 3)))
    w2_p = np.ascontiguousarray(np.transpose(w2, (2, 0, 1, 3)))
    return {
        "wqk": _np_bf16(wqk_p), "wv": _np_bf16(wv_p), "wo": _np_bf16(wo_p),
        "w1": _np_bf16(w1_p), "w2": _np_bf16(w2_p),
        "mk": _np_bf16(mk), "mq": _np_bf16(mq), "ident": _np_bf16(ident),
    }


def kernel(h, norm1_w, in_proj_w, in_proj_b, out_proj_w, out_proj_b,
           norm2_w, ff1_w, ff1_b, ff2_w, ff2_b):
    from concourse.bass_utils import run_bass_kernel_spmd

    h = np.asarray(h, np.float32)
    aux = prep_aux(norm1_w, in_proj_w, out_proj_w, norm2_w, ff1_w, ff2_w)

    key = ("nc", T)
    if key not in _BUILD_CACHE:
        _BUILD_CACHE[key] = build_nc(T)
    nc = _BUILD_CACHE[key]

    in_maps = []
    for c in range(N_CORES):
        m = {"x": np.ascontiguousarray(h[c])}
        m.update(aux)
        in_maps.append(m)

    res = run_bass_kernel_spmd(nc, in_maps, list(range(N_CORES)),
                               trace=bool(int(os.environ.get("KERNEL_TRACE", "0"))))
    if res.exec_time_ns is not None:
        kernel.last_exec_time_ns = res.exec_time_ns
    out = np.stack([res.results[c]["out"] for c in range(N_CORES)], axis=0)
    return out


kernel.last_exec_time_ns = None

